# revision 19
# baseline (speedup 1.0000x reference)
"""Sparse (chunked-causal | bidirectional-block) GQA attention on 8 trn2 cores.

Full inputs in, full output out. Sharding: core j handles batch b = j // 4 and
kv-heads {2*(j%4), 2*(j%4)+1} (= query heads 4*(j%4) .. 4*(j%4)+3).

Split of work:
  - The DEVICE computes attention over the static chunk-causal block
    structure (all 128x128 blocks (t, s) with kv-tile t <= q-tile s in the
    same chunk). Diagonal blocks are masked with batch-exact 0/1 masks
    (causal triangle + any bidirectional-run extras inside the tile);
    off-diagonal in-chunk blocks are always fully allowed.
  - Bidirectional runs that CROSS a 128-row tile boundary create a few
    extra, nearly-empty blocks off that structure. The q columns they touch
    (a handful per batch) are recomputed exactly on the HOST in fp32 and
    overwritten in the output.

The host also does all layout work so the device kernel is pure attention
math on DMA-friendly layouts: q/k cast to fp16 (q pre-scaled by 1/sqrt(D))
and pre-transposed to [d, s]; v cast to fp16 with a ones column appended
(softmax denominators fall out of the PV matmul); every DMA descriptor is
>=4KB contiguous per partition.

Per-core bass kernel, per (head, group-of-512-q) work item:
  - S^T[kv, q] via PE matmuls (lhsT = K^T tile, rhs = Q^T cols) into a
    PSUM tile; full-block pieces packed first, diagonal blocks last, with
    no piece crossing a PSUM bank boundary and every matmul a full
    128-partition tile_position=(0,0) op (uniform PE config, weight loads
    pipeline back-to-back).
  - one ACT exp per item -> E (fp16, SBUF).
  - one DVE multiply applies the packed diagonal masks (contiguous tail).
  - PV: per block, accumulate matmul lhsT=E-slice, rhs=V_aug tile; the
    ones column gives denominators. PV matmuls of the lagged item are
    interleaved between QK matmuls of the current item.
  - normalize: DVE reciprocal + Pool broadcast multiply into a
    4-head-interleaved out tile; one output DMA per group of 512 q rows.
"""

import math

import numpy as np

import concourse.bass as bass
import concourse.mybir as mybir
import concourse.tile as tile
from concourse import bacc
from concourse.bass_utils import run_bass_kernel_spmd

B, S, HQ, HKV, D = 2, 2048, 16, 8, 128
TS = 128                  # block tile size (partitions)
NT = S // TS              # 16 q/kv tiles
GROUP_SUBTILES = 4        # q-subtiles per group (512 q rows)
N_GROUPS = NT // GROUP_SUBTILES
BANK_COLS = 512           # fp32 cols per PSUM bank
ST_COLS = 1536            # st tile cols (3 banks; one group in one round)
N_CORES = 8
PAIRS_PER_CORE = 2        # kv heads per core
HEADS_PER_CORE = 4        # query heads per core

F16 = mybir.dt.float16
F32 = mybir.dt.float32


# ---------------------------------------------------------------- host masks

def _segment_ids(m):
    """[B, S] 0/1 -> contiguous-run segment ids (0 = not in a run)."""
    mm = m.astype(np.int64)
    padded = np.pad(mm, ((0, 0), (1, 0)))
    boundary = padded[:, 1:] > padded[:, :-1]
    return mm * np.cumsum(boundary, axis=1)


def _allowed_T(bidirectional_mask, chunk):
    """Per-batch allowed mask, transposed: [B, S(kv), S(q)] bool."""
    seg = _segment_ids(np.asarray(bidirectional_mask))
    r = np.arange(S)
    chunk_ok = (r[:, None] // chunk == r[None, :] // chunk) & (r[:, None] >= r[None, :])
    out = np.zeros((B, S, S), dtype=bool)
    for b in range(B):
        bid = (seg[b][:, None] == seg[b][None, :]) & (seg[b][:, None] > 0)
        out[b] = (chunk_ok | bid).T
    return out


class Schedule:
    """Device schedule over the static chunk-causal structure; any u_any
    block off that structure is deferred to the host (fix_cols).

    groups[g] = dict with fields:
      cols: total packed e-columns
      qk:   [(t, e_off, q_abs, n)]   matmul pieces, none crossing a bank
      mask: (e_lo, mbuf_off, w)      single DVE mask mult (diag tail)
      pv:   {s_local: [(t, e_off)]}  accumulation lists (all 128-wide)
    """

    def __init__(self, allowed_T, chunk):
        blocks = allowed_T.reshape(B, NT, TS, NT, TS)
        b_any = blocks.any(axis=(2, 4))
        u_any = b_any.any(axis=0)
        tpc = max(chunk // TS, 1)   # tiles per chunk
        tt, ss = np.meshgrid(np.arange(NT), np.arange(NT), indexing="ij")
        causal = (tt // tpc == ss // tpc) & (ss >= tt)

        # host-fix columns: q extents of any allowed block off the structure
        colmask = blocks.any(axis=(0, 2))  # [t, s, q_in_tile]
        fix = np.zeros(S, dtype=bool)
        for t in range(NT):
            for s in range(NT):
                if u_any[t, s] and not causal[t, s]:
                    fix[s * TS:(s + 1) * TS] |= colmask[t, s]
        self.fix_cols = np.nonzero(fix)[0]

        self.mask_slices = []   # ordered t of diag blocks -> host buffer
        mbuf_off = 0
        self.groups = []
        for g in range(N_GROUPS):
            s0 = g * GROUP_SUBTILES
            t_list = [t for t in range(NT)
                      if any(causal[t, s] for s in range(s0, s0 + GROUP_SUBTILES))]
            # full pieces: for t, span of s>t blocks in group; diag last
            fulls = []
            for t in t_list:
                ss_full = [s for s in range(s0, s0 + GROUP_SUBTILES)
                           if causal[t, s] and s != t]
                if ss_full:
                    lo, hi = min(ss_full), max(ss_full) + 1
                    assert ss_full == list(range(lo, hi))
                    fulls.append((t, lo, hi - lo))
            diags = [t for t in t_list if s0 <= t < s0 + GROUP_SUBTILES]

            work = {"cols": 0, "qk": [], "mask": None,
                    "pv": {sl: [] for sl in range(GROUP_SUBTILES)}}
            e_of_block = {}
            off = 0

            def place(t, q_abs, w):
                """Place a w-col piece; split so no part crosses a bank."""
                nonlocal off
                o = off
                rem, q = w, q_abs
                while rem > 0:
                    n = min(BANK_COLS - o % BANK_COLS, rem)
                    work["qk"].append((t, o, q, n))
                    o += n
                    q += n
                    rem -= n
                start = off
                off += w
                return start

            # first-fit-decreasing keeps full pieces bank-aligned
            for (t, lo, nsub) in sorted(fulls, key=lambda x: -x[2]):
                start = place(t, lo * TS, nsub * TS)
                for i in range(nsub):
                    e_of_block[(t, lo + i)] = start + i * TS
            mask_lo = off
            for t in diags:
                e_of_block[(t, t)] = place(t, t * TS, TS)
                self.mask_slices.append(t)
            if off > mask_lo:
                work["mask"] = (mask_lo, mbuf_off, off - mask_lo)
                mbuf_off += off - mask_lo
            work["cols"] = off
            assert off <= ST_COLS, f"group {g}: {off} cols > {ST_COLS}"

            for s in range(s0, s0 + GROUP_SUBTILES):
                for t in range(NT):
                    if causal[t, s]:
                        work["pv"][s - s0].append((t, e_of_block[(t, s)]))
            self.groups.append(work)

        self.n_mask_cols = mbuf_off

    def mask_data(self, allowed_T_b):
        """[TS, n_mask_cols] fp16 0/1 packed diag-mask buffer, one batch."""
        out = np.zeros((TS, max(self.n_mask_cols, 1)), dtype=np.float16)
        for i, t in enumerate(self.mask_slices):
            out[:, i * TS:(i + 1) * TS] = \
                allowed_T_b[t * TS:(t + 1) * TS, t * TS:(t + 1) * TS]
        return out

    def key(self):
        return (tuple(self.mask_slices),
                tuple((g["cols"], tuple(g["qk"])) for g in self.groups))


# ------------------------------------------------------------- kernel build

def _broadcast_free(ap, n):
    """Append a 0-step free dim of size n to an AP (read-broadcast)."""
    return bass.AP(tensor=ap.tensor, offset=ap.offset, ap=[*ap.ap, [0, n]])


def _split_dim(ap, n0, n1):
    """Split an AP's first free dim of size n0*n1 into (n0, n1)."""
    (pstep, pnum), (fstep, fnum), *rest = ap.ap
    assert fnum == n0 * n1
    return bass.AP(tensor=ap.tensor, offset=ap.offset,
                   ap=[[pstep, pnum], [fstep * n1, n0], [fstep, n1], *rest])


def _build_body(nc, tc, sched: Schedule, tensors, safe_pv=False):
    qT_in, kT_in, v_in, m_in, o_out = tensors
    ctxs = []
    pv_first_mms = []

    def pool(*a, **kw):
        p = tc.tile_pool(*a, **kw)
        ctxs.append(p)
        return p.__enter__()

    consts = pool(name="consts", bufs=1)
    ktp = pool(name="ktp", bufs=4 * PAIRS_PER_CORE)
    qtp = pool(name="qtp", bufs=4 * HEADS_PER_CORE)
    vp = pool(name="vp", bufs=1)
    epool = pool(name="epool", bufs=5)
    e2pool = pool(name="e2pool", bufs=5)
    outp = pool(name="outp", bufs=N_GROUPS)
    small = pool(name="small", bufs=4)
    stp = pool(name="st_psum", bufs=1 if safe_pv else 2, space="PSUM")
    pvp = pool(name="pv_psum", bufs=1, space="PSUM")

    nmask = max(sched.n_mask_cols, 1)
    mask_sb = consts.tile([TS, nmask], F16)

    # loads in group-sized quarters (512 cols), ordered to match the
    # group-major work order so the PE never starves on a late chunk
    QS = S // 4
    kts = [[None] * 4 for _ in range(PAIRS_PER_CORE)]
    qts = [[None] * 4 for _ in range(HEADS_PER_CORE)]

    def load_kt(pair, ci):
        t_ = ktp.tile([TS, QS], F16, tag="ktq")
        nc.sync.dma_start(out=t_, in_=kT_in[:, pair, ci * QS:(ci + 1) * QS])
        kts[pair][ci] = t_

    def load_qt(head, ci):
        t_ = qtp.tile([TS, QS], F16, tag="qtq")
        nc.sync.dma_start(out=t_, in_=qT_in[:, head, ci * QS:(ci + 1) * QS])
        qts[head][ci] = t_

    load_kt(0, 0)
    load_qt(0, 0)
    nc.sync.dma_start(out=mask_sb, in_=m_in[:, :])
    load_kt(1, 0)
    load_qt(1, 0)
    v_sb = vp.tile([TS, NT, PAIRS_PER_CORE, D + 1], F16, tag="v")
    nc.sync.dma_start(out=v_sb, in_=v_in[:, :, :, :])
    load_qt(2, 0)
    load_qt(3, 0)
    for ci in range(1, 4):
        load_kt(0, ci)
        load_qt(0, ci)
        load_kt(1, ci)
        load_qt(1, ci)
        load_qt(2, ci)
        load_qt(3, ci)

    def kt_slice(pair, t):
        ci, o = divmod(t * TS, QS)
        return kts[pair][ci][:, o:o + TS]

    def qt_slice(head, q0, n):
        ci, o = divmod(q0, QS)
        assert o + n <= QS
        return qts[head][ci][:, o:o + n]

    out_tiles = [outp.tile([TS, GROUP_SUBTILES, HEADS_PER_CORE, D], F16,
                           name=f"out_{g}", tag="out")
                 for g in range(N_GROUPS)]

    nbank = GROUP_SUBTILES if safe_pv else 2
    per = 1 if safe_pv else 2

    work = []
    for g in range(N_GROUPS):
        for head in range(HEADS_PER_CORE):
            work.append({"head": head, "pair": head // 2, "g": g,
                         "w": sched.groups[g]})

    def front_mms(w):
        gw = w["w"]
        st = stp.tile([TS, ST_COLS], F32, tag="st")
        w["st"] = st
        thunks = []
        for (t, e_off, q0, n) in gw["qk"]:
            def mk(t=t, e_off=e_off, q0=q0, n=n):
                nc.tensor.matmul(
                    st[:, e_off:e_off + n],
                    lhsT=kt_slice(w["pair"], t),
                    rhs=qt_slice(w["head"], q0, n),
                    start=True, stop=True,
                )
            thunks.append(mk)
        return thunks

    def front_tail(w):
        gw = w["w"]
        st = w["st"]
        e = epool.tile([TS, ST_COLS], F16, tag="e")
        nc.scalar.activation(
            e[:, 0:gw["cols"]], st[:, 0:gw["cols"]],
            mybir.ActivationFunctionType.Exp,
        )
        w["e"] = e
        w["e2"] = None
        if gw["mask"] is not None:
            # masked diag cols go to a separate tile so full-block PV
            # matmuls depend only on exp, not on the mask multiply
            (e_lo, moff, mw) = gw["mask"]
            e2 = e2pool.tile([TS, BANK_COLS], F16, tag="e2")
            nc.vector.tensor_mul(
                e2[:, 0:mw],
                e[:, e_lo:e_lo + mw],
                mask_sb[:, moff:moff + mw],
            )
            w["e2"] = e2

    def back_mms(w):
        gw, g, head, pair = w["w"], w["g"], w["head"], w["pair"]
        pv = pvp.tile([TS, nbank, per, BANK_COLS // per], F32,
                      name=f"pv_{head}_{g}", tag="pv")
        w["pv"] = pv
        e = w["e"]
        bank_first = [None] * nbank
        bank_mms = [[] for _ in range(nbank)]
        bank_total = [0] * nbank
        bank_done = [0] * nbank
        for sl in range(GROUP_SUBTILES):
            bank_total[sl // per] += len(gw["pv"][sl])
        mask_lo = gw["mask"][0] if gw["mask"] is not None else None
        full_thunks, diag_thunks = [], []
        for sl in range(GROUP_SUBTILES):
            bk, sub = divmod(sl, per)
            for (t, e_off) in gw["pv"][sl]:
                diag = mask_lo is not None and e_off >= mask_lo

                def mk(bk=bk, sub=sub, t=t, e_off=e_off, diag=diag):
                    src = (w["e2"][:, e_off - mask_lo:e_off - mask_lo + TS]
                           if diag else e[:, e_off:e_off + TS])
                    first = bank_first[bk] is None
                    bank_done[bk] += 1
                    mm = nc.tensor.matmul(
                        pv[:, bk, sub, 0:D + 1],
                        lhsT=src,
                        rhs=v_sb[:, t, pair, 0:D + 1],
                        start=first,
                        stop=bank_done[bk] == bank_total[bk],
                    )
                    if first:
                        bank_first[bk] = mm.ins.name
                    else:
                        bank_mms[bk].append(mm.ins.name)
                (diag_thunks if diag else full_thunks).append(mk)
        w["bank_state"] = (bank_first, bank_mms)
        return full_thunks + diag_thunks

    def back_tail(w):
        g, head = w["g"], w["head"]
        pv = w["pv"]
        (bank_first, bank_mms) = w["bank_state"]
        pv_first_mms.extend(
            (f, o) for f, o in zip(bank_first, bank_mms) if f is not None)
        recip = small.tile([TS, nbank, per], F32, tag="recip")
        nc.vector.reciprocal(recip, pv[:, :, :, D])
        out_t = out_tiles[g]
        out_ap = _split_dim(out_t[:, :, head, :], nbank, per)
        nc.vector.tensor_mul(out_ap, pv[:, :, :, 0:D],
                             _broadcast_free(recip, D))
        if head == HEADS_PER_CORE - 1:
            nc.sync.dma_start(out=o_out[:, g, :, :, :], in_=out_t)

    def interleave(a, b):
        if not b:
            return list(a)
        if not a:
            return list(b)
        out = []
        na, nb = len(a), len(b)
        ia = ib = 0
        while ia < na or ib < nb:
            if ia < na:
                out.append(a[ia])
                ia += 1
            while ib * na <= ia * nb and ib < nb:
                out.append(b[ib])
                ib += 1
        return out

    LAG = min(3, max(1, len(work) - 1))
    n = len(work)
    for i in range(n + LAG):
        fr = front_mms(work[i]) if i < n else []
        bk = back_mms(work[i - LAG]) if i >= LAG else []
        for thunk in interleave(fr, bk):
            thunk()
        # back_tail first: its recip/norm must precede the next mask in the
        # in-order DVE queue, else the pv-psum WAR stalls the PE stream
        if i >= LAG:
            back_tail(work[i - LAG])
        if i < n:
            front_tail(work[i])

    for p in reversed(ctxs):
        p.__exit__(None, None, None)
    return pv_first_mms


def _verify_pv_order(nc, pv_first_mms):
    pos = {}
    i = 0
    for bb in nc.m.functions[0].blocks:
        for ins in bb.instructions:
            pos[ins.name] = i
            i += 1
    for first, others in pv_first_mms:
        p0 = pos.get(first)
        if p0 is None:
            return False
        for o in others:
            po = pos.get(o)
            if po is None or po < p0:
                return False
    return True


def _build_kernel(sched: Schedule, safe_pv: bool = False):
    nc = bacc.Bacc("TRN2", target_bir_lowering=False, debug=False,
                   num_devices=N_CORES, name="sparse_attn")

    qT_in = nc.dram_tensor("qT", [TS, HEADS_PER_CORE, S], F16, kind="ExternalInput")
    kT_in = nc.dram_tensor("kT", [TS, PAIRS_PER_CORE, S], F16, kind="ExternalInput")
    v_in = nc.dram_tensor("vaug", [TS, NT, PAIRS_PER_CORE, D + 1], F16,
                          kind="ExternalInput")
    m_in = nc.dram_tensor("maskb", [TS, max(sched.n_mask_cols, 1)], F16,
                          kind="ExternalInput")
    o_out = nc.dram_tensor("o", [TS, N_GROUPS, GROUP_SUBTILES, HEADS_PER_CORE, D],
                           F16, kind="ExternalOutput")
    tensors = (qT_in, kT_in, v_in, m_in, o_out)

    with tile.TileContext(nc) as tc:
        pv_first_mms = _build_body(nc, tc, sched, tensors, safe_pv=safe_pv)

    nc.compile()
    if not safe_pv and not _verify_pv_order(nc, pv_first_mms):
        return _build_kernel(sched, safe_pv=True)
    return nc


# --------------------------------------------------------------- entry point

_CACHE = {}


def _get_kernel(sched: Schedule):
    key = sched.key()
    if key not in _CACHE:
        _CACHE[key] = _build_kernel(sched)
    return _CACHE[key]


def _shard_inputs(q, k, v, masks_f16):
    scale = 1.0 / math.sqrt(D)
    in_maps = []
    for core in range(N_CORES):
        b = core // 4
        m = core % 4
        qT = np.ascontiguousarray(
            (q[b, :, 4 * m:4 * m + 4, :] * scale).astype(np.float16)
            .transpose(2, 1, 0))                       # [D, 4, S]
        kT = np.ascontiguousarray(
            k[b, :, 2 * m:2 * m + 2, :].astype(np.float16)
            .transpose(2, 1, 0))                       # [D, 2, S]
        vc = v[b, :, 2 * m:2 * m + 2, :].astype(np.float16)
        vaug = np.ones((S, 2, D + 1), dtype=np.float16)
        vaug[:, :, :D] = vc
        vaug = np.ascontiguousarray(
            vaug.reshape(NT, TS, 2, D + 1).transpose(1, 0, 2, 3))
        in_maps.append({
            "qT": qT, "kT": kT, "vaug": vaug, "maskb": masks_f16[b],
        })
    return in_maps


def _host_fix(out, q, k, v, allowed_T, cols):
    """Recompute the given q columns exactly (fp32) and overwrite."""
    if len(cols) == 0:
        return
    scale = 1.0 / math.sqrt(D)
    group = HQ // HKV
    for b in range(B):
        qb = q[b, cols, :, :]                          # [R, HQ, D]
        al = allowed_T[b][:, cols].T                   # [R, S(kv)]
        # logits[r, hq, kv]
        kb = np.repeat(k[b], group, axis=1)            # [S, HQ, D]
        logits = np.einsum("rhd,shd->rhs", qb * scale, kb)
        logits = np.where(al[:, None, :], logits, -np.inf)
        mx = logits.max(axis=-1, keepdims=True)
        e = np.exp(logits - mx)
        p = e / e.sum(axis=-1, keepdims=True)
        vb = np.repeat(v[b], group, axis=1)            # [S, HQ, D]
        out[b, cols, :, :] = np.einsum("rhs,shd->rhd", p, vb)


def kernel(q, k, v, bidirectional_mask, chunk_size):
    q = np.asarray(q, dtype=np.float32)
    k = np.asarray(k, dtype=np.float32)
    v = np.asarray(v, dtype=np.float32)
    chunk = int(np.asarray(chunk_size))

    allowed_T = _allowed_T(bidirectional_mask, chunk)
    sched = Schedule(allowed_T, chunk)
    nc = _get_kernel(sched)

    masks_f16 = [sched.mask_data(allowed_T[b]) for b in range(B)]
    in_maps = _shard_inputs(q, k, v, masks_f16)

    res = run_bass_kernel_spmd(nc, in_maps, list(range(N_CORES)))

    out = np.empty((B, S, HQ, D), dtype=np.float32)
    for core in range(N_CORES):
        b = core // 4
        m = core % 4
        oc = res.results[core]["o"]      # [TS, N_GROUPS, GROUP_SUBTILES, 4, D]
        oc = oc.transpose(1, 2, 0, 3, 4).reshape(S, HEADS_PER_CORE, D)
        out[b, :, 4 * m:4 * m + 4, :] = oc.astype(np.float32)

    _host_fix(out, q, k, v, allowed_T, sched.fix_cols)
    return out


# revision 22
# speedup vs baseline: 1.0215x; 1.0215x over previous
"""Sparse (chunked-causal | bidirectional-block) GQA attention on 8 trn2 cores.

Full inputs in, full output out. Sharding: core j handles batch b = j // 4 and
kv-heads {2*(j%4), 2*(j%4)+1} (= query heads 4*(j%4) .. 4*(j%4)+3).

Split of work:
  - The DEVICE computes attention over the static chunk-causal block
    structure (all 128x128 blocks (t, s) with kv-tile t <= q-tile s in the
    same chunk). Diagonal blocks are masked with batch-exact 0/1 masks
    (causal triangle + any bidirectional-run extras inside the tile);
    off-diagonal in-chunk blocks are always fully allowed.
  - Bidirectional runs that CROSS a 128-row tile boundary create a few
    extra, nearly-empty blocks off that structure. The q columns they touch
    (a handful per batch) are recomputed exactly on the HOST in fp32 and
    overwritten in the output.

The host also does all layout work so the device kernel is pure attention
math on DMA-friendly layouts: q/k cast to fp16 (q pre-scaled by 1/sqrt(D))
and pre-transposed to [d, s]; v cast to fp16 with a ones column appended
(softmax denominators fall out of the PV matmul); every DMA descriptor is
>=4KB contiguous per partition.

Per-core bass kernel, per (head, group-of-512-q) work item:
  - S^T[kv, q] via PE matmuls (lhsT = K^T tile, rhs = Q^T cols) into a
    PSUM tile; full-block pieces packed first, diagonal blocks last, with
    no piece crossing a PSUM bank boundary and every matmul a full
    128-partition tile_position=(0,0) op (uniform PE config, weight loads
    pipeline back-to-back).
  - one ACT exp per item -> E (fp16, SBUF).
  - one DVE multiply applies the packed diagonal masks (contiguous tail).
  - PV: per block, accumulate matmul lhsT=E-slice, rhs=V_aug tile; the
    ones column gives denominators. PV matmuls of the lagged item are
    interleaved between QK matmuls of the current item.
  - normalize: DVE reciprocal + Pool broadcast multiply into a
    4-head-interleaved out tile; one output DMA per group of 512 q rows.
"""

import math

import numpy as np

import concourse.bass as bass
import concourse.mybir as mybir
import concourse.tile as tile
from concourse import bacc
from concourse.bass_utils import run_bass_kernel_spmd

B, S, HQ, HKV, D = 2, 2048, 16, 8, 128
TS = 128                  # block tile size (partitions)
NT = S // TS              # 16 q/kv tiles
GROUP_SUBTILES = 4        # q-subtiles per group (512 q rows)
N_GROUPS = NT // GROUP_SUBTILES
BANK_COLS = 512           # fp32 cols per PSUM bank
ST_COLS = 1536            # st tile cols (3 banks; one group in one round)
N_CORES = 8
PAIRS_PER_CORE = 2        # kv heads per core
HEADS_PER_CORE = 4        # query heads per core

F16 = mybir.dt.float16
F32 = mybir.dt.float32


# ---------------------------------------------------------------- host masks

def _segment_ids(m):
    """[B, S] 0/1 -> contiguous-run segment ids (0 = not in a run)."""
    mm = m.astype(np.int64)
    padded = np.pad(mm, ((0, 0), (1, 0)))
    boundary = padded[:, 1:] > padded[:, :-1]
    return mm * np.cumsum(boundary, axis=1)


def _allowed_T(bidirectional_mask, chunk):
    """Per-batch allowed mask, transposed: [B, S(kv), S(q)] bool."""
    seg = _segment_ids(np.asarray(bidirectional_mask))
    r = np.arange(S)
    chunk_ok = (r[:, None] // chunk == r[None, :] // chunk) & (r[:, None] >= r[None, :])
    out = np.zeros((B, S, S), dtype=bool)
    for b in range(B):
        bid = (seg[b][:, None] == seg[b][None, :]) & (seg[b][:, None] > 0)
        out[b] = (chunk_ok | bid).T
    return out


class Schedule:
    """Device schedule over the static chunk-causal structure; any u_any
    block off that structure is deferred to the host (fix_cols).

    groups[g] = dict with fields:
      cols: total packed e-columns
      qk:   [(t, e_off, q_abs, n)]   matmul pieces, none crossing a bank
      mask: (e_lo, mbuf_off, w)      single DVE mask mult (diag tail)
      pv:   {s_local: [(t, e_off)]}  accumulation lists (all 128-wide)
    """

    def __init__(self, allowed_T, chunk):
        blocks = allowed_T.reshape(B, NT, TS, NT, TS)
        b_any = blocks.any(axis=(2, 4))
        u_any = b_any.any(axis=0)
        tpc = max(chunk // TS, 1)   # tiles per chunk
        tt, ss = np.meshgrid(np.arange(NT), np.arange(NT), indexing="ij")
        causal = (tt // tpc == ss // tpc) & (ss >= tt)

        # host-fix columns: q extents of any allowed block off the structure
        colmask = blocks.any(axis=(0, 2))  # [t, s, q_in_tile]
        fix = np.zeros(S, dtype=bool)
        for t in range(NT):
            for s in range(NT):
                if u_any[t, s] and not causal[t, s]:
                    fix[s * TS:(s + 1) * TS] |= colmask[t, s]
        self.fix_cols = np.nonzero(fix)[0]

        self.mask_slices = []   # ordered t of diag blocks -> host buffer
        mbuf_off = 0
        self.groups = []
        for g in range(N_GROUPS):
            s0 = g * GROUP_SUBTILES
            t_list = [t for t in range(NT)
                      if any(causal[t, s] for s in range(s0, s0 + GROUP_SUBTILES))]
            # full pieces: for t, span of s>t blocks in group; diag last
            fulls = []
            for t in t_list:
                ss_full = [s for s in range(s0, s0 + GROUP_SUBTILES)
                           if causal[t, s] and s != t]
                if ss_full:
                    lo, hi = min(ss_full), max(ss_full) + 1
                    assert ss_full == list(range(lo, hi))
                    fulls.append((t, lo, hi - lo))
            diags = [t for t in t_list if s0 <= t < s0 + GROUP_SUBTILES]

            work = {"cols": 0, "qk": [], "mask": None,
                    "pv": {sl: [] for sl in range(GROUP_SUBTILES)}}
            e_of_block = {}
            off = 0

            def place(t, q_abs, w):
                """Place a w-col piece; split so no part crosses a bank."""
                nonlocal off
                o = off
                rem, q = w, q_abs
                while rem > 0:
                    n = min(BANK_COLS - o % BANK_COLS, rem)
                    work["qk"].append((t, o, q, n))
                    o += n
                    q += n
                    rem -= n
                start = off
                off += w
                return start

            # first-fit-decreasing keeps full pieces bank-aligned
            for (t, lo, nsub) in sorted(fulls, key=lambda x: -x[2]):
                start = place(t, lo * TS, nsub * TS)
                for i in range(nsub):
                    e_of_block[(t, lo + i)] = start + i * TS
            mask_lo = off
            for t in diags:
                e_of_block[(t, t)] = place(t, t * TS, TS)
                self.mask_slices.append(t)
            if off > mask_lo:
                work["mask"] = (mask_lo, mbuf_off, off - mask_lo)
                mbuf_off += off - mask_lo
            work["cols"] = off
            assert off <= ST_COLS, f"group {g}: {off} cols > {ST_COLS}"

            for s in range(s0, s0 + GROUP_SUBTILES):
                for t in range(NT):
                    if causal[t, s]:
                        work["pv"][s - s0].append((t, e_of_block[(t, s)]))
            self.groups.append(work)

        self.n_mask_cols = mbuf_off

    def mask_data(self, allowed_T_b):
        """[TS, n_mask_cols] fp16 0/1 packed diag-mask buffer, one batch."""
        out = np.zeros((TS, max(self.n_mask_cols, 1)), dtype=np.float16)
        for i, t in enumerate(self.mask_slices):
            out[:, i * TS:(i + 1) * TS] = \
                allowed_T_b[t * TS:(t + 1) * TS, t * TS:(t + 1) * TS]
        return out

    def key(self):
        return (tuple(self.mask_slices),
                tuple((g["cols"], tuple(g["qk"])) for g in self.groups))


# ------------------------------------------------------------- kernel build

def _broadcast_free(ap, n):
    """Append a 0-step free dim of size n to an AP (read-broadcast)."""
    return bass.AP(tensor=ap.tensor, offset=ap.offset, ap=[*ap.ap, [0, n]])


def _split_dim(ap, n0, n1):
    """Split an AP's first free dim of size n0*n1 into (n0, n1)."""
    (pstep, pnum), (fstep, fnum), *rest = ap.ap
    assert fnum == n0 * n1
    return bass.AP(tensor=ap.tensor, offset=ap.offset,
                   ap=[[pstep, pnum], [fstep * n1, n0], [fstep, n1], *rest])


def _build_body(nc, tc, sched: Schedule, tensors, safe_pv=False):
    qT_in, kT_in, v_in, m_in, o_out = tensors
    ctxs = []
    pv_first_mms = []

    def pool(*a, **kw):
        p = tc.tile_pool(*a, **kw)
        ctxs.append(p)
        return p.__enter__()

    consts = pool(name="consts", bufs=1)
    ktp = pool(name="ktp", bufs=4 * PAIRS_PER_CORE)
    qtp = pool(name="qtp", bufs=4 * HEADS_PER_CORE)
    vp = pool(name="vp", bufs=1)
    epool = pool(name="epool", bufs=5)
    e2pool = pool(name="e2pool", bufs=5)
    outp = pool(name="outp", bufs=N_GROUPS)
    small = pool(name="small", bufs=4)
    stp = pool(name="st_psum", bufs=1 if safe_pv else 2, space="PSUM")
    pvp = pool(name="pv_psum", bufs=1, space="PSUM")

    nmask = max(sched.n_mask_cols, 1)
    mask_sb = consts.tile([TS, nmask], F16)

    # loads in group-sized quarters (512 cols), ordered to match the
    # group-major work order so the PE never starves on a late chunk
    QS = S // 4
    kts = [[None] * 4 for _ in range(PAIRS_PER_CORE)]
    qts = [[None] * 4 for _ in range(HEADS_PER_CORE)]

    def load_kt(pair, ci, eng=None):
        t_ = ktp.tile([TS, QS], F16, tag="ktq")
        (eng or nc.sync).dma_start(out=t_, in_=kT_in[:, pair, ci * QS:(ci + 1) * QS])
        kts[pair][ci] = t_

    def load_qt(head, ci, eng=None):
        t_ = qtp.tile([TS, QS], F16, tag="qtq")
        (eng or nc.sync).dma_start(out=t_, in_=qT_in[:, head, ci * QS:(ci + 1) * QS])
        qts[head][ci] = t_

    # spread the startup issues across the scalar sequencer (idle until the
    # first exp) so the sync queue's per-issue cost doesn't serialize the
    # first-quarter loads
    load_kt(0, 0)
    load_qt(0, 0, nc.scalar)
    nc.scalar.dma_start(out=mask_sb, in_=m_in[:, :])
    load_kt(1, 0)
    load_qt(1, 0, nc.scalar)
    v_sb = vp.tile([TS, NT, PAIRS_PER_CORE, D + 1], F16, tag="v")
    nc.sync.dma_start(out=v_sb, in_=v_in[:, :, :, :])
    load_qt(2, 0, nc.scalar)
    load_qt(3, 0, nc.scalar)
    for ci in range(1, 4):
        load_kt(0, ci)
        load_qt(0, ci)
        load_kt(1, ci)
        load_qt(1, ci)
        load_qt(2, ci)
        load_qt(3, ci)

    def kt_slice(pair, t):
        ci, o = divmod(t * TS, QS)
        return kts[pair][ci][:, o:o + TS]

    def qt_slice(head, q0, n):
        ci, o = divmod(q0, QS)
        assert o + n <= QS
        return qts[head][ci][:, o:o + n]

    out_tiles = [outp.tile([TS, GROUP_SUBTILES, HEADS_PER_CORE, D], F16,
                           name=f"out_{g}", tag="out")
                 for g in range(N_GROUPS)]

    nbank = GROUP_SUBTILES if safe_pv else 2
    per = 1 if safe_pv else 2

    work = []
    for g in range(N_GROUPS):
        for head in range(HEADS_PER_CORE):
            work.append({"head": head, "pair": head // 2, "g": g,
                         "w": sched.groups[g]})

    def front_mms(w):
        gw = w["w"]
        st = stp.tile([TS, ST_COLS], F32, tag="st")
        w["st"] = st
        thunks = []
        for (t, e_off, q0, n) in gw["qk"]:
            def mk(t=t, e_off=e_off, q0=q0, n=n):
                nc.tensor.matmul(
                    st[:, e_off:e_off + n],
                    lhsT=kt_slice(w["pair"], t),
                    rhs=qt_slice(w["head"], q0, n),
                    start=True, stop=True,
                )
            thunks.append(mk)
        return thunks

    def front_tail(w):
        gw = w["w"]
        st = w["st"]
        e = epool.tile([TS, ST_COLS], F16, tag="e")
        nc.scalar.activation(
            e[:, 0:gw["cols"]], st[:, 0:gw["cols"]],
            mybir.ActivationFunctionType.Exp,
        )
        w["e"] = e
        w["e2"] = None
        if gw["mask"] is not None:
            # masked diag cols go to a separate tile so full-block PV
            # matmuls depend only on exp, not on the mask multiply
            (e_lo, moff, mw) = gw["mask"]
            e2 = e2pool.tile([TS, BANK_COLS], F16, tag="e2")
            nc.vector.tensor_mul(
                e2[:, 0:mw],
                e[:, e_lo:e_lo + mw],
                mask_sb[:, moff:moff + mw],
            )
            w["e2"] = e2

    def back_mms(w):
        gw, g, head, pair = w["w"], w["g"], w["head"], w["pair"]
        pv = pvp.tile([TS, nbank, per, BANK_COLS // per], F32,
                      name=f"pv_{head}_{g}", tag="pv")
        w["pv"] = pv
        e = w["e"]
        bank_first = [None] * nbank
        bank_mms = [[] for _ in range(nbank)]
        bank_total = [0] * nbank
        bank_done = [0] * nbank
        for sl in range(GROUP_SUBTILES):
            bank_total[sl // per] += len(gw["pv"][sl])
        mask_lo = gw["mask"][0] if gw["mask"] is not None else None
        full_thunks, diag_thunks = [], []
        for sl in range(GROUP_SUBTILES):
            bk, sub = divmod(sl, per)
            for (t, e_off) in gw["pv"][sl]:
                diag = mask_lo is not None and e_off >= mask_lo

                def mk(bk=bk, sub=sub, t=t, e_off=e_off, diag=diag):
                    src = (w["e2"][:, e_off - mask_lo:e_off - mask_lo + TS]
                           if diag else e[:, e_off:e_off + TS])
                    first = bank_first[bk] is None
                    bank_done[bk] += 1
                    mm = nc.tensor.matmul(
                        pv[:, bk, sub, 0:D + 1],
                        lhsT=src,
                        rhs=v_sb[:, t, pair, 0:D + 1],
                        start=first,
                        stop=bank_done[bk] == bank_total[bk],
                    )
                    if first:
                        bank_first[bk] = mm.ins.name
                    else:
                        bank_mms[bk].append(mm.ins.name)
                (diag_thunks if diag else full_thunks).append(mk)
        w["bank_state"] = (bank_first, bank_mms)
        return full_thunks + diag_thunks

    def back_tail(w):
        g, head = w["g"], w["head"]
        pv = w["pv"]
        (bank_first, bank_mms) = w["bank_state"]
        pv_first_mms.extend(
            (f, o) for f, o in zip(bank_first, bank_mms) if f is not None)
        recip = small.tile([TS, nbank, per], F32, tag="recip")
        nc.vector.reciprocal(recip, pv[:, :, :, D])
        out_t = out_tiles[g]
        out_ap = _split_dim(out_t[:, :, head, :], nbank, per)
        nc.vector.tensor_mul(out_ap, pv[:, :, :, 0:D],
                             _broadcast_free(recip, D))
        if head == HEADS_PER_CORE - 1:
            nc.sync.dma_start(out=o_out[:, g, :, :, :], in_=out_t)

    def interleave(a, b):
        if not b:
            return list(a)
        if not a:
            return list(b)
        out = []
        na, nb = len(a), len(b)
        ia = ib = 0
        while ia < na or ib < nb:
            if ia < na:
                out.append(a[ia])
                ia += 1
            while ib * na <= ia * nb and ib < nb:
                out.append(b[ib])
                ib += 1
        return out

    LAG = min(2, max(1, len(work) - 1))
    n = len(work)
    for i in range(n + LAG):
        fr = front_mms(work[i]) if i < n else []
        bk = back_mms(work[i - LAG]) if i >= LAG else []
        for thunk in interleave(fr, bk):
            thunk()
        # back_tail first: its recip/norm must precede the next mask in the
        # in-order DVE queue, else the pv-psum WAR stalls the PE stream
        if i >= LAG:
            back_tail(work[i - LAG])
        if i < n:
            front_tail(work[i])

    for p in reversed(ctxs):
        p.__exit__(None, None, None)
    return pv_first_mms


def _verify_pv_order(nc, pv_first_mms):
    pos = {}
    i = 0
    for bb in nc.m.functions[0].blocks:
        for ins in bb.instructions:
            pos[ins.name] = i
            i += 1
    for first, others in pv_first_mms:
        p0 = pos.get(first)
        if p0 is None:
            return False
        for o in others:
            po = pos.get(o)
            if po is None or po < p0:
                return False
    return True


def _build_kernel(sched: Schedule, safe_pv: bool = False):
    nc = bacc.Bacc("TRN2", target_bir_lowering=False, debug=False,
                   num_devices=N_CORES, name="sparse_attn")

    qT_in = nc.dram_tensor("qT", [TS, HEADS_PER_CORE, S], F16, kind="ExternalInput")
    kT_in = nc.dram_tensor("kT", [TS, PAIRS_PER_CORE, S], F16, kind="ExternalInput")
    v_in = nc.dram_tensor("vaug", [TS, NT, PAIRS_PER_CORE, D + 1], F16,
                          kind="ExternalInput")
    m_in = nc.dram_tensor("maskb", [TS, max(sched.n_mask_cols, 1)], F16,
                          kind="ExternalInput")
    o_out = nc.dram_tensor("o", [TS, N_GROUPS, GROUP_SUBTILES, HEADS_PER_CORE, D],
                           F16, kind="ExternalOutput")
    tensors = (qT_in, kT_in, v_in, m_in, o_out)

    with tile.TileContext(nc) as tc:
        pv_first_mms = _build_body(nc, tc, sched, tensors, safe_pv=safe_pv)

    nc.compile()
    if not safe_pv and not _verify_pv_order(nc, pv_first_mms):
        return _build_kernel(sched, safe_pv=True)
    return nc


# --------------------------------------------------------------- entry point

_CACHE = {}


def _get_kernel(sched: Schedule):
    key = sched.key()
    if key not in _CACHE:
        _CACHE[key] = _build_kernel(sched)
    return _CACHE[key]


def _shard_inputs(q, k, v, masks_f16):
    scale = 1.0 / math.sqrt(D)
    in_maps = []
    for core in range(N_CORES):
        b = core // 4
        m = core % 4
        qT = np.ascontiguousarray(
            (q[b, :, 4 * m:4 * m + 4, :] * scale).astype(np.float16)
            .transpose(2, 1, 0))                       # [D, 4, S]
        kT = np.ascontiguousarray(
            k[b, :, 2 * m:2 * m + 2, :].astype(np.float16)
            .transpose(2, 1, 0))                       # [D, 2, S]
        vc = v[b, :, 2 * m:2 * m + 2, :].astype(np.float16)
        vaug = np.ones((S, 2, D + 1), dtype=np.float16)
        vaug[:, :, :D] = vc
        vaug = np.ascontiguousarray(
            vaug.reshape(NT, TS, 2, D + 1).transpose(1, 0, 2, 3))
        in_maps.append({
            "qT": qT, "kT": kT, "vaug": vaug, "maskb": masks_f16[b],
        })
    return in_maps


def _host_fix(out, q, k, v, allowed_T, cols):
    """Recompute the given q columns exactly (fp32) and overwrite."""
    if len(cols) == 0:
        return
    scale = 1.0 / math.sqrt(D)
    group = HQ // HKV
    for b in range(B):
        qb = q[b, cols, :, :]                          # [R, HQ, D]
        al = allowed_T[b][:, cols].T                   # [R, S(kv)]
        # logits[r, hq, kv]
        kb = np.repeat(k[b], group, axis=1)            # [S, HQ, D]
        logits = np.einsum("rhd,shd->rhs", qb * scale, kb)
        logits = np.where(al[:, None, :], logits, -np.inf)
        mx = logits.max(axis=-1, keepdims=True)
        e = np.exp(logits - mx)
        p = e / e.sum(axis=-1, keepdims=True)
        vb = np.repeat(v[b], group, axis=1)            # [S, HQ, D]
        out[b, cols, :, :] = np.einsum("rhs,shd->rhd", p, vb)


def kernel(q, k, v, bidirectional_mask, chunk_size):
    q = np.asarray(q, dtype=np.float32)
    k = np.asarray(k, dtype=np.float32)
    v = np.asarray(v, dtype=np.float32)
    chunk = int(np.asarray(chunk_size))

    allowed_T = _allowed_T(bidirectional_mask, chunk)
    sched = Schedule(allowed_T, chunk)
    nc = _get_kernel(sched)

    masks_f16 = [sched.mask_data(allowed_T[b]) for b in range(B)]
    in_maps = _shard_inputs(q, k, v, masks_f16)

    res = run_bass_kernel_spmd(nc, in_maps, list(range(N_CORES)))

    out = np.empty((B, S, HQ, D), dtype=np.float32)
    for core in range(N_CORES):
        b = core // 4
        m = core % 4
        oc = res.results[core]["o"]      # [TS, N_GROUPS, GROUP_SUBTILES, 4, D]
        oc = oc.transpose(1, 2, 0, 3, 4).reshape(S, HEADS_PER_CORE, D)
        out[b, :, 4 * m:4 * m + 4, :] = oc.astype(np.float32)

    _host_fix(out, q, k, v, allowed_T, sched.fix_cols)
    return out
